# revision 60
# baseline (speedup 1.0000x reference)
"""EnvelopeDetector Trainium2 kernel (Bass/Tile), batch-sharded over 8
NeuronCores. Each core owns 4 of the 32 batch rows for ALL 64 channels;
BatchNorm uses per-core local batch stats (sync-free approximation over
4x19901 = 79,604 samples/channel, well within tolerance).

Host/dispatch design (the steady-state wall-clock is dominated by host
passes + host<->device transfer, not device exec):
  - x ships in NATURAL [B, C, T] layout as bf16: the only host-side pass
    over the data is one astype(bf16). Per-core shard = contiguous slice.
  - z returns in NATURAL [B, C, T2] layout (global concat of per-core
    [4, 64, T2] shards IS the final array): no host reassembly, only one
    bf16->f32 cast pass.
  - Weight-derived constants (Toeplitz band matrices, identity, ones,
    scalar table) are cached on device across calls keyed by the raw
    weight bytes: zero per-call upload cost in steady state.
  - The donated output buffer rolls: each call's result buffer is donated
    as the next call's output scratch, so no zero-buffer upload and no
    per-call zeros dispatch (one tiny on-device zeros jit on call 1).

Device dataflow (5-stage software pipeline over 64 channels). The
(j,b)-partition shear between natural [b, t] DRAM layout and the
transpose-ready SBUF packing is done with batched DRAM->DRAM DMAs
(tile-framework dep tracking is blind to partition-split SBUF views, and
batching 64 channels into ~20 DMAs per direction amortizes per-DMA
overheads); the shear and un-shear run in channel-quarters interleaved
with the pipeline so they overlap compute:
  load : one plain DMA of the channel's pre-sheared tile
         x4[4j+b, 128g+u] = x[b, 4096g+128j+u].
  txs  : 5 PE transposes -> x_T[u, 4m+b] = x[b, 128m+u]  (chunk m<160).
  front: conv1 (depthwise K=100) as PE matmuls with 128x128 Toeplitz
         stationaries A1/B1; moving = x_T windows; fp32 PSUM; evacuation
         to bf16 y_T with fused per-partition sum accumulation
         (accum_out); sum-of-squares straight from PSUM on ACT (Square +
         accum_out); exact-region partials for the tail chunk (u < 61).
  mid  : ones-matmul partition-reduce of stats; scalar chain ->
         scale = gamma/std, b' = (beta/gamma)*std - mean, using
         |s*y + bias| = s*|y + b'| (s > 0); PE broadcast; one wide ACT
         Abs -> bf16 a_T.
  back : conv2 (K=50): stationary = 128-col a_T blocks, moving = A2/B2;
         each 128-col PSUM region its own accumulation group; evacuation
         applies z = s*psum + b_low into bf16 zt in natural [b, t] order;
         staged to the channel's zsh slot for the batched un-shear.
"""

import sys

import numpy as np

try:
    import concourse.bass as bass  # noqa: F401
except ImportError:  # pragma: no cover
    sys.path.insert(0, "/opt/trn_rl_repo")

B, C, T = 32, 64, 20000
TP = 20480  # host zero-pads x to 4096*5 so the strided load is uniform
K1, K2 = 100, 50
T1 = T - K1 + 1  # 19901
T2 = T1 - K2 + 1  # 19852
NCORES = 8
BL = B // NCORES  # 4 batches per core
BN_EPS = 1e-5

P = 128
XCOLS = 5 * P  # 640: x_T chunks 0..159 (t zero-padded past 20000)
YCOLS = 624  # y_T chunks 0..155 (chunk 155 valid only for u < 61)
ACOLS = 648  # a_T + zero pad for conv2's shifted stationary windows

_CACHE = {}


def _build_program():
    import concourse.tile as tile
    from concourse import bacc, mybir
    from contextlib import ExitStack

    f32 = mybir.dt.float32
    bf16 = mybir.dt.bfloat16
    AFT = mybir.ActivationFunctionType
    ALU = mybir.AluOpType
    AX = mybir.AxisListType

    nc = bacc.Bacc("TRN2", target_bir_lowering=False, debug=False,
                   num_devices=NCORES)

    x_d = nc.dram_tensor("x_loc", [BL, C, TP], bf16,
                         kind="ExternalInput").ap()
    tp_d = nc.dram_tensor("toep", [P, C * 2 * P], bf16,
                          kind="ExternalInput").ap()
    tp2_d = nc.dram_tensor("toep2", [P, C * 2 * P], bf16,
                           kind="ExternalInput").ap()
    cb_d = nc.dram_tensor("cb", [4, C], f32, kind="ExternalInput").ap()
    id_d = nc.dram_tensor("ident", [P, P], bf16, kind="ExternalInput").ap()
    on_d = nc.dram_tensor("ones", [P, P], f32, kind="ExternalInput").ap()
    z_d = nc.dram_tensor("z_loc", [BL, C, T2], bf16,
                         kind="ExternalOutput").ap()
    # DRAM staging for the (j,b) partition shear (tile dep tracking is
    # blind to partition-split SBUF views, and DRAM APs are unrestricted):
    # addr = p*(C*640) + c*640 + col, p = 4j+b, col = 128g+u
    xsh = nc.dram_tensor("xsh", [P, C * XCOLS], bf16).ap()
    zsh = nc.dram_tensor("zsh", [P, C * 5 * P], bf16).ap()

    NTOT = float(BL * T1)

    with tile.TileContext(nc) as tc:
        with ExitStack() as ctx:
            p_const = ctx.enter_context(tc.tile_pool(name="const", bufs=1))
            p_x4 = ctx.enter_context(tc.tile_pool(name="x4", bufs=4))
            p_xt = ctx.enter_context(tc.tile_pool(name="xt", bufs=3))
            p_yt = ctx.enter_context(tc.tile_pool(name="yt", bufs=3))
            p_at = ctx.enter_context(tc.tile_pool(name="at", bufs=3))
            p_zt = ctx.enter_context(tc.tile_pool(name="zt", bufs=3))
            p_st = ctx.enter_context(tc.tile_pool(name="st", bufs=3))
            p_sq = ctx.enter_context(tc.tile_pool(name="sq", bufs=2))
            pp_tx = ctx.enter_context(
                tc.tile_pool(name="pptx", bufs=2, space="PSUM"))
            pp_y = ctx.enter_context(
                tc.tile_pool(name="ppy", bufs=2, space="PSUM"))
            pp_y2 = ctx.enter_context(
                tc.tile_pool(name="ppy2", bufs=1, space="PSUM"))
            pp_z = ctx.enter_context(
                tc.tile_pool(name="ppz", bufs=1, space="PSUM"))
            pp_zb = ctx.enter_context(
                tc.tile_pool(name="ppzb", bufs=1, space="PSUM"))
            pp_m = ctx.enter_context(
                tc.tile_pool(name="ppm", bufs=1, space="PSUM"))

            # ---- constants (toeplitz pre-transposed on host: the DMA is
            # 128 contiguous 32KB rows instead of 16K strided descriptors) --
            toep_sb = p_const.tile([P, C * 2 * P], bf16, tag="toep")
            nc.sync.dma_start(toep_sb[:], tp_d)
            toep2_sb = p_const.tile([P, C * 2 * P], bf16, tag="toep2")
            nc.sync.dma_start(toep2_sb[:], tp2_d)
            id_sb = p_const.tile([P, P], bf16, tag="ident")
            nc.sync.dma_start(id_sb[:], id_d)
            on_sb = p_const.tile([P, P], f32, tag="ones")
            nc.sync.dma_start(on_sb[:], on_d)
            cb_sb = p_const.tile([1, 4 * C], f32, tag="cb")
            nc.sync.dma_start(cb_sb[:], cb_d.flatten().unsqueeze(0))
            # broadcast b_low for all channels once: [128, C]
            pmb = pp_m.tile([P, C], f32, tag="m")
            nc.tensor.matmul(pmb[:], on_sb[0:1, :], cb_sb[0:1, 2 * C:3 * C])
            blow_bc = p_const.tile([P, C], f32, tag="blow")
            nc.vector.tensor_copy(blow_bc[:], pmb[:])
            eps_sb = p_const.tile([1, 1], f32, tag="eps")
            nc.vector.memset(eps_sb[:], BN_EPS)

            # ---- batched DRAM->DRAM x shear (channel-quarters) ----
            # xsh[p, 640c + 128g + u] = x[b, c, 4096g + 128j + u], p = 4j+b
            xshv = xsh.rearrange("p (c col) -> p c col", c=C, col=XCOLS)
            xshv4 = xshv.rearrange("(j b) c col -> j b c col", j=32, b=BL)
            CH = C // 4

            def shear_x(c0, cn):
                for b in range(BL):
                    for g in range(5):
                        nc.sync.dma_start(
                            xshv4[:, b, c0:c0 + cn,
                                  P * g:P * (g + 1)].rearrange(
                                "j c u -> c j u"),
                            x_d[b, c0:c0 + cn,
                                4096 * g:4096 * (g + 1)].rearrange(
                                "c (j u) -> c j u", j=32, u=P))

            shear_x(0, CH)

            def load(c):
                """One plain DMA of channel c's pre-sheared tile into SBUF:
                x4[4j+b, 128g+u] = x[b, 4096g+128j+u] (host zero-pads to
                t=20480)."""
                t4 = p_x4.tile([P, XCOLS], bf16, tag="x4")
                nc.sync.dma_start(t4[:], xshv[:, c, :])
                return t4

            def txs(c, t4):
                """PE transposes -> x_T[u, 4m+b] (chunk m = 32g+j)."""
                xt = p_xt.tile([P, XCOLS], bf16, tag="xt")
                ptx = pp_tx.tile([P, XCOLS], bf16, tag="tx")
                for g in range(5):
                    nc.tensor.transpose(ptx[:, P * g:P * (g + 1)],
                                        t4[:, P * g:P * (g + 1)], id_sb[:])
                nc.vector.tensor_copy(xt[:], ptx[:])
                return xt

            def front(c, xt):
                """conv1 + local BN stats accumulation for channel c."""
                A1 = toep_sb[:, (2 * c + 0) * P:(2 * c + 1) * P]
                B1 = toep_sb[:, (2 * c + 1) * P:(2 * c + 2) * P]
                yt = p_yt.tile([P, YCOLS + 16], bf16, tag="yt")
                # statcols: 0 sum-bank0, 1 sum-bank1-main, 2 sum-tail-partial,
                #           3 sumsq-main, 4 sumsq-tail-partial
                statcols = p_st.tile([P, 8], f32, tag="statcols")
                nc.vector.memset(statcols[:], 0.0)
                py0 = pp_y.tile([P, 512], f32, tag="y0")
                nc.tensor.matmul(py0[:], A1, xt[:, 0:512],
                                 start=True, stop=False)
                nc.tensor.matmul(py0[:], B1, xt[:, 4:516],
                                 start=False, stop=True)
                py1 = pp_y2.tile([P, P], f32, tag="y1")
                nc.tensor.matmul(py1[:, 0:112], A1, xt[:, 512:624],
                                 start=True, stop=False)
                nc.tensor.matmul(py1[:, 0:112], B1, xt[:, 516:628],
                                 start=False, stop=True)
                # evacuate with fused per-partition sums
                nc.vector.tensor_scalar(
                    yt[:, 0:512], py0[:], 0.0, 0.0, op0=ALU.add, op1=ALU.add,
                    accum_out=statcols[:, 0:1])
                nc.vector.tensor_scalar(
                    yt[:, 512:620], py1[:, 0:108], 0.0, 0.0,
                    op0=ALU.add, op1=ALU.add, accum_out=statcols[:, 1:2])
                # tail chunk 155 (cols 620:624): valid only u < 61
                nc.vector.tensor_copy(yt[:, 620:624], py1[:, 108:112])
                nc.vector.tensor_scalar(
                    yt[0:61, 624:628], py1[0:61, 108:112], 0.0, 0.0,
                    op0=ALU.add, op1=ALU.add, accum_out=statcols[0:61, 2:3])
                # sum-of-squares straight from PSUM (ACT engine, f32;
                # runs concurrently with the DVE evacuations)
                sq = p_sq.tile([P, YCOLS], f32, tag="sq")
                nc.scalar.activation(sq[:, 0:512], py0[:], AFT.Square,
                                     accum_out=statcols[:, 3:4])
                nc.scalar.activation(sq[:, 512:620], py1[:, 0:108], AFT.Square,
                                     accum_out=statcols[:, 5:6])
                nc.scalar.activation(sq[0:61, 620:624], py1[0:61, 108:112],
                                     AFT.Square, accum_out=statcols[0:61, 4:5])
                return {"yt": yt, "statcols": statcols}

            def mid(c, stt):
                """BN local-stats scalar chain + |scale*y + bias|."""
                yt, statcols = stt["yt"], stt["statcols"]
                at = p_at.tile([P, ACOLS], bf16, tag="at")
                pm = pp_m.tile([P, 32], f32, tag="m")
                nc.tensor.matmul(pm[0:1, 0:8], on_sb[:, 0:1], statcols[:])
                ss = p_st.tile([1, 2], f32, tag="ss")
                nc.vector.reduce_sum(ss[:, 0:1], pm[0:1, 0:3], axis=AX.X)
                nc.vector.reduce_sum(ss[:, 1:2], pm[0:1, 3:6], axis=AX.X)
                mE = p_st.tile([1, 2], f32, tag="mE")
                nc.vector.tensor_scalar_mul(mE[:], ss[:], 1.0 / NTOT)
                msq = p_st.tile([1, 1], f32, tag="msq")
                nc.vector.tensor_mul(msq[:], mE[:, 0:1], mE[:, 0:1])
                var = p_st.tile([1, 1], f32, tag="var")
                nc.vector.tensor_sub(var[:], mE[:, 1:2], msq[:])
                s0 = p_st.tile([1, 1], f32, tag="s0")
                nc.scalar.activation(s0[:], var[:], AFT.Sqrt, bias=eps_sb[:])
                inv = p_st.tile([1, 1], f32, tag="inv")
                nc.vector.reciprocal(inv[:], s0[:])
                # sb3: [scale = gamma/std, b' = (beta/gamma)*std - mean]
                sb3 = p_st.tile([1, 2], f32, tag="sb3")
                nc.vector.tensor_mul(sb3[:, 0:1], inv[:], cb_sb[:, c:c + 1])
                nc.vector.scalar_tensor_tensor(
                    sb3[:, 1:2], s0[:], cb_sb[:, 3 * C + c:3 * C + c + 1],
                    mE[:, 0:1], op0=ALU.mult, op1=ALU.subtract)
                nc.tensor.matmul(pm[:, 8:10], on_sb[0:1, :], sb3[:])
                bc = p_st.tile([P, 2], f32, tag="bcast")
                nc.vector.tensor_copy(bc[:], pm[:, 8:10])

                # a' = |y + b'| -> bf16 a_T; zero the conv2 pad region
                nc.vector.memset(at[:, YCOLS:ACOLS], 0.0)
                nc.scalar.activation(at[:, 0:YCOLS], yt[:, 0:YCOLS],
                                     AFT.Abs, bias=bc[:, 1:2])
                return {"at": at, "bc": bc}

            def back(c, stt):
                """conv2 + scale + b_low bias + strided store."""
                at, bc = stt["at"], stt["bc"]
                A2 = toep2_sb[:, (2 * c + 0) * P:(2 * c + 1) * P]
                B2 = toep2_sb[:, (2 * c + 1) * P:(2 * c + 2) * P]
                blv = blow_bc[:, c:c + 1]
                zt = p_zt.tile([P, 5 * P], bf16, tag="zt")

                # bank A: z chunk blocks s=0..3 (chunks 32s..32s+31); each
                # 128-col region is its own accumulation group (same
                # pattern as the per-region PE transposes).
                pz = pp_z.tile([P, 512], f32, tag="z")
                for s in range(4):
                    out_ap = pz[:, P * s:P * (s + 1)]
                    nc.tensor.matmul(out_ap, at[:, P * s:P * s + P], A2,
                                     start=True, stop=False,
                                     skip_group_check=True)
                    nc.tensor.matmul(out_ap, at[:, P * s + 4:P * s + 132], B2,
                                     start=False, stop=True,
                                     skip_group_check=True)
                # bank B: chunks 128..155 (single region)
                pzB = pp_zb.tile([P, P], f32, tag="zB")
                nc.tensor.matmul(pzB[:], at[:, 512:640], A2,
                                 start=True, stop=False)
                nc.tensor.matmul(pzB[:], at[:, 516:644], B2,
                                 start=False, stop=True)

                nc.vector.tensor_scalar(zt[:, 0:512], pz[:], bc[:, 0:1], blv,
                                        op0=ALU.mult, op1=ALU.add)
                nc.scalar.activation(zt[:, 512:640], pzB[:], AFT.Identity,
                                     bias=blv, scale=bc[:, 0:1])

                # stage to the channel's zsh slot; un-sheared in one
                # batched pass after the pipeline
                nc.gpsimd.dma_start(
                    zsh.rearrange("p (c col) -> p c col",
                                  c=C, col=5 * P)[:, c, :],
                    zt[:])

            # batched DRAM->DRAM z un-shear, one channel-quarter at a time:
            # z[b, c, 128(32s+j) + u] = zsh[4j+b, 640c + 128s + u]
            zshv4 = zsh.rearrange("(j b) (c col) -> j b c col",
                                  j=32, b=BL, c=C, col=5 * P)

            def unshear_z(c0, cn):
                # alternate DGE units per batch row so descriptor
                # generation for the un-shear runs on both in parallel
                for b in range(BL):
                    eng = nc.gpsimd if b % 2 else nc.sync
                    for s in range(4):
                        eng.dma_start(
                            z_d[b, c0:c0 + cn,
                                4096 * s:4096 * (s + 1)].rearrange(
                                "c (j u) -> c j u", j=32, u=P),
                            zshv4[:, b, c0:c0 + cn,
                                  P * s:P * (s + 1)].rearrange(
                                "j c u -> c j u"))
                    eng.dma_start(
                        z_d[b, c0:c0 + cn, 16384:19840].rearrange(
                            "c (j u) -> c j u", j=27, u=P),
                        zshv4[0:27, b, c0:c0 + cn, 512:640].rearrange(
                            "j c u -> c j u"))
                    eng.dma_start(
                        z_d[b, c0:c0 + cn, 19840:19852],
                        zshv4[27, b, c0:c0 + cn, 512:524])

            # 5-stage software pipeline across channels; later x-shear
            # quarters are issued while earlier ones stream, and each z
            # un-shear quarter as soon as its last channel has stored.
            lds, txd, frs, mds = {}, {}, {}, {}
            for c in range(C + 4):
                if c >= 2 and (c - 2) % CH == 0 and (c - 2) // CH + 1 < 4:
                    shear_x(CH * ((c - 2) // CH + 1), CH)
                if c < C:
                    lds[c] = load(c)
                if c >= 4:
                    back(c - 4, mds.pop(c - 4))
                    if (c - 3) % CH == 0 and 1 <= (c - 3) // CH <= 3:
                        unshear_z(CH * ((c - 3) // CH - 1), CH)
                    elif c - 4 == C - 9:
                        unshear_z(C - CH, CH // 2)
                    elif c - 4 == C - 1:
                        unshear_z(C - CH // 2, CH // 2)
                if 3 <= c <= C + 2:
                    mds[c - 3] = mid(c - 3, frs.pop(c - 3))
                if 2 <= c <= C + 1:
                    frs[c - 2] = front(c - 2, txd.pop(c - 2))
                if 1 <= c <= C:
                    txd[c - 1] = txs(c - 1, lds.pop(c - 1))

    nc.compile()
    return nc


def _toeplitz_consts(w_band, w_low, gamma, beta, b_low):
    """Host-built weight-derived constant arrays (small; built once per
    distinct weight bytes and cached on device)."""
    import ml_dtypes
    bf16 = ml_dtypes.bfloat16
    wb = np.asarray(w_band, dtype=np.float32).reshape(C, K1)
    wl = np.asarray(w_low, dtype=np.float32).reshape(C, K2)
    gamma = np.asarray(gamma, dtype=np.float32).reshape(C)
    beta = np.asarray(beta, dtype=np.float32).reshape(C)
    b_low = np.asarray(b_low, dtype=np.float32).reshape(C)

    v = np.arange(P)[:, None]
    m = np.arange(P)[None, :]

    def toep_pair(w, K):
        dA = v - m
        dB = v + P - m
        A = np.where((dA >= 0) & (dA < K), w[:, np.clip(dA, 0, K - 1)], 0.0)
        Bm = np.where((dB >= 0) & (dB < K), w[:, np.clip(dB, 0, K - 1)], 0.0)
        return A, Bm

    A1, B1 = toep_pair(wb, K1)
    A2, B2 = toep_pair(wl, K2)

    def pretrans(Am, Bm):
        # [P, C*2*P]: row p holds [A[c][p-th row... stationary column p]
        s = np.stack([Am, Bm], axis=1)  # [C, 2, P(v), P(u)]
        return np.ascontiguousarray(
            s.transpose(2, 0, 1, 3).reshape(P, C * 2 * P)).astype(bf16)

    toep = pretrans(A1, B1)
    toep2 = pretrans(A2, B2)
    cb = np.ascontiguousarray(
        np.stack([gamma, beta, b_low,
                  beta / np.where(gamma != 0.0, gamma, 1.0)]))
    ident = np.eye(P, dtype=bf16)
    ones = np.ones((P, P), dtype=np.float32)
    return {"toep": toep, "toep2": toep2, "cb": cb,
            "ident": ident, "ones": ones}


def _get_exec():
    """Build (once) the bass program + the jitted sharded executable."""
    if "exec" in _CACHE:
        return _CACHE["exec"]
    import jax
    import jax.numpy as jnp
    from jax.sharding import Mesh, PartitionSpec, NamedSharding
    from jax.experimental.shard_map import shard_map
    from concourse import mybir
    from concourse.bass2jax import (_bass_exec_p, install_neuronx_cc_hook,
                                    partition_id_tensor)

    nc = _build_program()
    install_neuronx_cc_hook()

    partition_name = (nc.partition_id_tensor.name
                      if nc.partition_id_tensor else None)
    in_names, out_names, out_avals = [], [], []
    for alloc in nc.m.functions[0].allocations:
        if not isinstance(alloc, mybir.MemoryLocationSet):
            continue
        name = alloc.memorylocations[0].name
        if alloc.kind == "ExternalInput":
            if name != partition_name:
                in_names.append(name)
        elif alloc.kind == "ExternalOutput":
            out_names.append(name)
            out_avals.append(jax.core.ShapedArray(
                tuple(alloc.tensor_shape), mybir.dt.np(alloc.dtype)))
    n_params = len(in_names)
    all_in_names = list(in_names) + list(out_names)
    if partition_name is not None:
        all_in_names.append(partition_name)

    def _body(*args):
        operands = list(args)
        if partition_name is not None:
            operands.append(partition_id_tensor())
        outs = _bass_exec_p.bind(
            *operands,
            out_avals=tuple(out_avals),
            in_names=tuple(all_in_names),
            out_names=tuple(out_names),
            lowering_input_output_aliases=(),
            sim_require_finite=True,
            sim_require_nnan=True,
            nc=nc,
        )
        return tuple(outs)

    devices = jax.devices()[:NCORES]
    mesh = Mesh(np.asarray(devices), ("core",))
    shard = NamedSharding(mesh, PartitionSpec("core"))
    _CACHE["devices"] = devices
    n_in = n_params + len(out_names)
    sharded = jax.jit(
        shard_map(_body, mesh=mesh,
                  in_specs=(PartitionSpec("core"),) * n_in,
                  out_specs=(PartitionSpec("core"),) * len(out_names)),
        donate_argnums=tuple(range(n_params, n_in)),
        keep_unused=True,
    )
    zeros_fn = jax.jit(
        lambda: jnp.zeros((B, C, T2), jnp.bfloat16), out_shardings=shard)
    _CACHE["exec"] = {
        "nc": nc, "sharded": sharded, "zeros_fn": zeros_fn,
        "in_names": in_names, "shard": shard, "jax": jax,
    }
    return _CACHE["exec"]


def _device_consts(ex, w_band, w_low, gamma, beta, b_low):
    """Device-resident weight constants, cached keyed by raw bytes."""
    key = (np.asarray(w_band).tobytes(), np.asarray(w_low).tobytes(),
           np.asarray(gamma).tobytes(), np.asarray(beta).tobytes(),
           np.asarray(b_low).tobytes())
    cached = _CACHE.get("consts")
    if cached is not None and cached[0] == key:
        return cached[1]
    jax = ex["jax"]
    host = _toeplitz_consts(w_band, w_low, gamma, beta, b_low)
    # replicate over cores along axis 0 (shard_map shards axis 0)
    dev = {}
    for name, arr in host.items():
        rep = np.ascontiguousarray(
            np.broadcast_to(arr[None], (NCORES,) + arr.shape).reshape(
                (NCORES * arr.shape[0],) + arr.shape[1:]))
        dev[name] = jax.device_put(rep, ex["shard"])
    for a in dev.values():
        a.block_until_ready()
    _CACHE["consts"] = (key, dev)
    return dev


def run(inputs, trace=False):
    """Run on 8 NeuronCores; returns (z_full, exec_time_ns_or_None)."""
    import ml_dtypes
    ex = _get_exec()
    x = np.asarray(inputs["x"])
    xb = np.zeros((B, C, TP), ml_dtypes.bfloat16)
    xb[:, :, :T] = x  # single host pass: f32 -> bf16 cast into padded buffer
    consts = _device_consts(ex, inputs["w_band"], inputs["w_low"],
                            inputs["gamma"], inputs["beta"], inputs["b_low"])
    zbuf = _CACHE.pop("zbuf", None)
    if zbuf is None:
        zbuf = ex["zeros_fn"]()
    args = [xb if n == "x_loc" else consts[n] for n in ex["in_names"]]
    (zdev,) = ex["sharded"](*args, zbuf)
    z16 = np.asarray(zdev)
    _CACHE["zbuf"] = zdev
    return z16.astype(np.float32), None


def kernel(**inputs):
    z, _ = run(inputs)
    return z
